# revision 8
# baseline (speedup 1.0000x reference)
"""Trainium2 Bass kernel for nn_AugementationAttention.

Reference computation (per batch b of 16, N=512, D=4096, NH=16, DK=256):
    q = x @ Wq.T, k = x @ Wk.T, v = x @ Wv.T          (per-head dk=256)
    ep = aug @ Wp.T + bp                               (per-head n=512 == 2*dk)
    dist = softmax(q k^T / sqrt(dk) + ep, axis=-1)
    out  = dist @ v                                    -> (b, n, d)

Sharding: data-parallel over batch, 2 batch elements per core on 8 cores.

Per-core kernel structure (single fused pass over (batch, head)):
  - x^T and aug^T for the core's 2 batches stay resident in SBUF (16 MB).
  - Per head: stream this head's weight slices from HBM once, compute
    Q^T/K^T (dout-major) and V (n-major) projections as three separate
    32-k-tile accumulation sub-loops (so only 2 PSUM banks are live per
    sub-loop), then accumulate S^T[key, query] = (aug Wp^T)^T + (QK^T)^T
    in 4 PSUM banks, exp on ScalarE with the bp bias folded in.
  - The A@V stage of head h is deferred until after head h+1's projection
    matmuls (software pipelining) so the PE never waits on ScalarE's exp.
  - A ones-column appended to V makes column 256 of the A@V PSUM tile the
    softmax denominator; normalization happens in the PSUM->SBUF copy
    (vector reciprocal + per-partition tensor_scalar multiply).
  - Scores stay in the transposed [key, query] layout throughout, so no
    on-chip transposes are needed anywhere.
  - All matmuls run as float32r (full PE rate for free dims >= 256).
"""

import sys

sys.path.insert(0, "/opt/trn_rl_repo")

import numpy as np

import concourse.bacc as bacc
import concourse.mybir as mybir
import concourse.tile as tile
from concourse.bass_utils import run_bass_kernel_spmd

F32 = mybir.dt.float32
F32R = mybir.dt.float32r

B, N, D, NH, DK = 16, 512, 4096, 16, 256
NCORES = 8
BL = B // NCORES  # batches per core
KT = D // 128  # 32 k-tiles of the contraction dim
G = 4  # k-tiles per weight DMA chunk
NG = KT // G  # chunks per head per projection
SCALE = 1.0 / np.sqrt(DK)

def _build_program():
    nc = bacc.Bacc(
        "TRN2",
        target_bir_lowering=False,
        debug=False,
        enable_asserts=False,
        num_devices=NCORES,
    )

    xt = nc.dram_tensor("xt", [BL, 128, KT, N], F32R, kind="ExternalInput")
    at = nc.dram_tensor("at", [BL, 128, KT, N], F32R, kind="ExternalInput")
    # w*[h, g, p, G*dout]: per chunk g, k-tile j in chunk, dout cols of head h
    wq = nc.dram_tensor("wq", [NH, NG, 128, G * 256], F32R, kind="ExternalInput")
    wk = nc.dram_tensor("wk", [NH, NG, 128, G * 256], F32R, kind="ExternalInput")
    wv = nc.dram_tensor("wv", [NH, NG, 128, G * 256], F32R, kind="ExternalInput")
    wp = nc.dram_tensor("wp", [NH, NG, 128, G * 512], F32R, kind="ExternalInput")
    bias = nc.dram_tensor("bias", [128, 64], F32, kind="ExternalInput")
    out = nc.dram_tensor("out", [BL, N, D], F32, kind="ExternalOutput")

    with tile.TileContext(nc) as tc:
        with (
            tc.tile_pool(name="const", bufs=1) as const_pool,
            tc.tile_pool(name="acts", bufs=1) as act_pool,
            tc.tile_pool(name="wgt", bufs=2) as w_pool,
            tc.tile_pool(name="qk", bufs=1) as qk_pool,
            tc.tile_pool(name="vv", bufs=2) as v_pool,
            tc.tile_pool(name="ee", bufs=2) as e_pool,
            tc.tile_pool(name="oo", bufs=2) as o_pool,
            tc.tile_pool(name="rr", bufs=8) as r_pool,
            tc.tile_pool(name="psp", bufs=4, space="PSUM") as ps_proj,
            tc.tile_pool(name="pss", bufs=4, space="PSUM") as ps_s,
        ):
            bias_sb = const_pool.tile([128, 64], F32)
            nc.sync.dma_start(out=bias_sb[:], in_=bias[:])

            def proj_qk(w_dram, h, xt_sb, name):
                """Q^T/K^T projection: psum[dt] [128 dout, N] over 32 k-tiles."""
                ps = [
                    ps_proj.tile([128, N], F32, tag="pp", name=f"ps{name}{i}")
                    for i in range(2)
                ]
                for g in range(NG):
                    wt = w_pool.tile([128, G, 256], F32R, tag="wqk", name=f"w{name}")
                    nc.sync.dma_start(out=wt[:], in_=w_dram[h, g])
                    for j in range(G):
                        kt = g * G + j
                        st, sp = kt == 0, kt == KT - 1
                        xk = (xt_sb[:, kt, :])
                        for dt in range(2):
                            nc.tensor.matmul(
                                ps[dt][:],
                                (wt[:, j, dt * 128 : (dt + 1) * 128]),
                                xk,
                                start=st,
                                stop=sp,
                            )
                sb = qk_pool.tile([128, 2, N], F32R, tag=f"{name}t", name=f"{name}t_sb")
                for dt in range(2):
                    nc.vector.tensor_copy(sb[:, dt, :], ps[dt][:])
                return sb

            def proj_v(h, xt_sb):
                """V projection (n-major): psum tiles pack 2 n-tiles per bank."""
                ps = [
                    ps_proj.tile([128, N], F32, tag="pp", name=f"psv{i}")
                    for i in range(2)
                ]
                for g in range(NG):
                    wt = w_pool.tile([128, G, 256], F32R, tag="wqk", name="wv")
                    nc.sync.dma_start(out=wt[:], in_=wv[h, g])
                    for j in range(G):
                        kt = g * G + j
                        wvj = (wt[:, j, :])
                        for nt in range(4):
                            # one accumulation group per PSUM bank: start only
                            # zeroes the whole 2KB zero region once, stop on the
                            # bank's last matmul
                            nc.tensor.matmul(
                                ps[nt // 2][:, (nt % 2) * 256 : (nt % 2 + 1) * 256],
                                (xt_sb[:, kt, nt * 128 : (nt + 1) * 128]),
                                wvj,
                                start=(kt == 0 and nt % 2 == 0),
                                stop=(kt == KT - 1 and nt % 2 == 1),
                            )
                v_sb = v_pool.tile([128, 4, 264], F32R, tag="v", name="v_sb")
                for nt in range(4):
                    nc.vector.tensor_copy(
                        v_sb[:, nt, 0:256],
                        ps[nt // 2][:, (nt % 2) * 256 : (nt % 2 + 1) * 256],
                    )
                nc.vector.memset(v_sb[:, :, 256:257].bitcast(F32), 1.0)
                nc.vector.memset(v_sb[:, :, 257:258].bitcast(F32), 0.0)
                return v_sb

            def attn_out(state):
                """Deferred A@V + normalize + store for a previous head."""
                if state is None:
                    return
                b, h, e_sb, v_sb = state
                ot_sb = o_pool.tile([128, 4, 256], F32, tag="ot", name="ot_sb")
                for qt in range(4):
                    pso = ps_s.tile([128, 258], F32, tag="ss", name="pso")
                    for jt in range(4):
                        nc.tensor.matmul(
                            pso[:],
                            (e_sb[:, jt, qt * 128 : (qt + 1) * 128]),
                            (v_sb[:, jt, 0:258]),
                            start=(jt == 0),
                            stop=(jt == 3),
                        )
                    r = r_pool.tile([128, 1], F32, tag="r", name="r")
                    nc.vector.reciprocal(r[:], pso[:, 256:257])
                    nc.vector.tensor_scalar_mul(ot_sb[:, qt, :], pso[:, 0:256], r[:])
                nc.sync.dma_start(
                    out=out[b].rearrange("(qt p) d -> p qt d", p=128)[
                        :, :, h * 256 : (h + 1) * 256
                    ],
                    in_=ot_sb[:],
                )

            pending = None
            for b in range(BL):
                xt_sb = act_pool.tile([128, KT, N], F32R, tag="xt", name="xt_sb")
                nc.sync.dma_start(out=xt_sb[:], in_=xt[b])
                at_sb = act_pool.tile([128, KT, N], F32R, tag="at", name="at_sb")
                nc.sync.dma_start(out=at_sb[:], in_=at[b])

                for h in range(NH):
                    qt_sb = proj_qk(wq, h, xt_sb, "q")
                    kt_sb = proj_qk(wk, h, xt_sb, "k")
                    v_sb = proj_v(h, xt_sb)

                    # previous head's A@V runs here: by now ScalarE has had a
                    # whole projection phase to finish the previous exp.
                    attn_out(pending)

                    # S^T[j, q] accumulation: Wp-projection + QK^T
                    pss = [
                        ps_s.tile([128, N], F32, tag="ss", name=f"pss{i}")
                        for i in range(4)
                    ]
                    for g in range(NG):
                        wt = w_pool.tile([128, G, 512], F32R, tag="wp", name="wpt")
                        nc.sync.dma_start(out=wt[:], in_=wp[h, g])
                        for j in range(G):
                            kt = g * G + j
                            ak = (at_sb[:, kt, :])
                            for jt in range(4):
                                nc.tensor.matmul(
                                    pss[jt][:],
                                    (wt[:, j, jt * 128 : (jt + 1) * 128]),
                                    ak,
                                    start=(kt == 0),
                                    stop=False,
                                )
                    for jt in range(4):
                        for dt in range(2):
                            nc.tensor.matmul(
                                pss[jt][:],
                                (kt_sb[:, dt, jt * 128 : (jt + 1) * 128]),
                                (qt_sb[:, dt, :]),
                                start=False,
                                stop=(dt == 1),
                            )

                    e_sb = e_pool.tile([128, 4, N], F32R, tag="e", name="e_sb")
                    for jt in range(4):
                        nc.scalar.activation(
                            e_sb[:, jt, :],
                            pss[jt][:],
                            mybir.ActivationFunctionType.Exp,
                            bias=bias_sb[:, h * 4 + jt : h * 4 + jt + 1],
                        )
                    pending = (b, h, e_sb, v_sb)

            attn_out(pending)

    nc.compile()
    return nc


_NC_CACHE = None


def _get_program():
    global _NC_CACHE
    if _NC_CACHE is None:
        _NC_CACHE = _build_program()
    return _NC_CACHE


def _pack_inputs(x, Augementation_embedding, Wq, Wk, Wv, Wp, bp):
    """Host-side relayout: transposes and per-head tiling, all in numpy."""
    f = np.float32
    x = np.asarray(x, f)
    aug = np.asarray(Augementation_embedding, f)

    # [B, N, D] -> [B, 128, KT, N] : k-tiled transpose
    def act_pack(a):
        t = a.transpose(0, 2, 1).reshape(B, KT, 128, N).transpose(0, 2, 1, 3)
        return np.ascontiguousarray(t)

    xt = act_pack(x)
    at = act_pack(aug)

    # W.T [D, dout_total] -> [NH, NG, 128, G*dout_per_head]
    def w_pack(w_t, dout_per_head):
        nh = w_t.shape[1] // dout_per_head
        t = w_t.reshape(KT, 128, nh, dout_per_head).transpose(2, 0, 1, 3)
        t = (
            t.reshape(nh, NG, G, 128, dout_per_head)
            .transpose(0, 1, 3, 2, 4)
            .reshape(nh, NG, 128, G * dout_per_head)
        )
        return np.ascontiguousarray(t)

    wq_pk = w_pack(np.asarray(Wq, f).T * np.float32(SCALE), 256)
    wk_pk = w_pack(np.asarray(Wk, f).T, 256)
    wv_pk = w_pack(np.asarray(Wv, f).T, 256)
    wp_pk = w_pack(np.asarray(Wp, f).T, 512)

    bias = np.ascontiguousarray(np.asarray(bp, f).reshape(64, 128).T)  # [128, 64]

    return xt, at, wq_pk, wk_pk, wv_pk, wp_pk, bias


def kernel(x, Augementation_embedding, Wq, Wk, Wv, Wp, bp):
    nc = _get_program()
    xt, at, wq_pk, wk_pk, wv_pk, wp_pk, bias = _pack_inputs(
        x, Augementation_embedding, Wq, Wk, Wv, Wp, bp
    )

    in_maps = []
    for c in range(NCORES):
        in_maps.append(
            {
                "xt": xt[c * BL : (c + 1) * BL],
                "at": at[c * BL : (c + 1) * BL],
                "wq": wq_pk,
                "wk": wk_pk,
                "wv": wv_pk,
                "wp": wp_pk,
                "bias": bias,
            }
        )

    res = run_bass_kernel_spmd(nc, in_maps, core_ids=list(range(NCORES)))
    outs = [res.results[c]["out"] for c in range(NCORES)]
    return np.concatenate(outs, axis=0).astype(np.float32)


if __name__ == "__main__":
    rng = np.random.default_rng(0)
    ins = {
        "x": rng.standard_normal((B, N, D), dtype=np.float32),
        "Augementation_embedding": rng.standard_normal((B, N, D), dtype=np.float32),
        "Wq": rng.standard_normal((D, D), dtype=np.float32) / np.sqrt(D),
        "Wk": rng.standard_normal((D, D), dtype=np.float32) / np.sqrt(D),
        "Wv": rng.standard_normal((D, D), dtype=np.float32) / np.sqrt(D),
        "Wp": rng.standard_normal((2 * D, D), dtype=np.float32) / np.sqrt(D),
        "bp": (rng.standard_normal(2 * D, dtype=np.float32) * 0.01),
    }
    o = kernel(**ins)
    print("out", o.shape, o.dtype, float(np.abs(o).max()))
